# revision 1
# baseline (speedup 1.0000x reference)
"""nn_CosineSimilarity GNN edge kernel for 8x TRN2 NeuronCores.

Strategy (sharding_hint: shard edges across cores, replicate node table):
  - Host: group edges into 16 buckets by (src>>15, dst>>15) so int16 gather
    indices reach their table chunk; sort each bucket by src for HBM
    locality; deal each bucket evenly to the 8 cores (identical per-core
    bucket sizes -> one SPMD program); pad per-core buckets to multiples
    of 128.
  - Device (per core): dma_gather 512B rows of h for src and dst of each
    edge tile; cos = dot(hs,hd) * rsqrt(sum(hs^2)*sum(hd^2)) computed with
    DVE (mult + segmented reduces), ACT (squares, sqrt), all overlapped
    with the gathers by the Tile framework.
  - Host: inverse-permute per-core padded outputs back to edge order.
"""

import sys

sys.path.insert(0, '/opt/trn_rl_repo')

import numpy as np

import concourse.bacc as bacc
import concourse.bass as bass
import concourse.mybir as mybir
from concourse.tile import TileContext
from concourse import bass_utils, library_config
from concourse.dve_ops import DveOp, OPS, _SUB_OPCODE_FOR_NAME, _CUSTOM_DVE_ROW_BASE
from concourse.dve_spec import Spec, Src0, Src1, scan, AluOp as DveAluOp, lower as dve_lower
from concourse.dve_uop import DveOpSpec


def _ref_mul_scan(in0, in1, c0, c1, c2):
    prod = in0.astype(np.float32) * in1.astype(np.float32)
    return np.cumsum(prod.reshape(in0.shape[0], -1), axis=-1).reshape(in0.shape)


def _make_mul_scan():
    """Fused out[p,k] = cumsum_k(in0*in1): one DVE pass for the edge dot
    products (segment ends differenced afterwards)."""
    if "MUL_SCAN" in _SUB_OPCODE_FOR_NAME:
        return next(o for o in OPS if o.name == "MUL_SCAN")
    spec = Spec(body=scan(DveAluOp.ADD, Src0 * Src1), reference=_ref_mul_scan)
    opcode = _CUSTOM_DVE_ROW_BASE + len(OPS)
    assert opcode < 0x20
    shas = {}
    for ver in ("v3", "v4"):
        try:
            tmp = DveOpSpec(name="MUL_SCAN", opcode=opcode,
                            uops=dve_lower(spec, ver=ver), rd1_en=True)
            shas[ver] = tmp.sha(ver)
        except Exception:
            pass
    op = DveOp("MUL_SCAN", spec, subdim=False, uops_sha=shas)
    OPS.append(op)
    _SUB_OPCODE_FOR_NAME[op.name] = opcode
    return op


MUL_SCAN = _make_mul_scan()

N, D, E = 100000, 128, 640000
NCORES = 8
P = 128
CH = 32768          # table chunk rows addressable by int16 gather indices
NCHUNK = (N + CH - 1) // CH
G = 2048            # max edges per dma_gather
SQ_DT = mybir.dt.float32   # dtype of squared tiles (fp16 halves DVE reduce time)


def _plan(src, dst):
    """Bucket/sort/shard edges. Returns per-core index arrays, tile list and
    the padded-position -> global-edge-id map."""
    src = np.asarray(src).astype(np.int64).ravel()
    dst = np.asarray(dst).astype(np.int64).ravel()
    e = src.shape[0]
    a = src >> 15
    b = dst >> 15
    key = a * NCHUNK + b
    order = np.lexsort((src, key))          # bucket-major, src-sorted inside
    key_sorted = key[order]
    bucket_starts = np.searchsorted(key_sorted, np.arange(NCHUNK * NCHUNK))
    bucket_ends = np.searchsorted(key_sorted, np.arange(NCHUNK * NCHUNK), side='right')

    # per-core padded layout (identical across cores)
    tiles = []          # (chunk_a, chunk_b, col16_off, gsize)
    btot = 0            # padded edges per core
    bucket_core_meta = []  # (g, start, end, B_g)
    for g in range(NCHUNK * NCHUNK):
        s0, s1 = int(bucket_starts[g]), int(bucket_ends[g])
        cnt = s1 - s0
        if cnt == 0:
            continue
        percore = -(-cnt // NCORES)             # ceil
        B_g = -(-percore // P) * P              # pad to multiple of 128
        bucket_core_meta.append((g, s0, s1, B_g))
        off = 0
        while off < B_g:
            gsz = min(G, B_g - off)
            tiles.append((g // NCHUNK, g % NCHUNK, btot + off, gsz))
            off += gsz
        btot += B_g

    sidx = np.zeros((NCORES, btot), np.int16)
    didx = np.zeros((NCORES, btot), np.int16)
    gid = np.full((NCORES, btot), -1, np.int64)

    pos = 0
    for g, s0, s1, B_g in bucket_core_meta:
        cnt = s1 - s0
        idxs = order[s0:s1]
        # contiguous near-equal slices keep the src sort per core
        splits = np.linspace(0, cnt, NCORES + 1).astype(np.int64)
        ca, cb = g // NCHUNK, g % NCHUNK
        for c in range(NCORES):
            sl = idxs[splits[c]:splits[c + 1]]
            n = sl.shape[0]
            sidx[c, pos:pos + n] = (src[sl] - ca * CH).astype(np.int16)
            didx[c, pos:pos + n] = (dst[sl] - cb * CH).astype(np.int16)
            gid[c, pos:pos + n] = sl
        pos += B_g
    assert pos == btot

    def wrap16(arr):            # [NCORES, btot] -> [NCORES, 128, btot//16]
        w = arr.reshape(NCORES, btot // 16, 16).transpose(0, 2, 1)  # [C,16,btot/16]
        return np.tile(w, (1, 8, 1)).astype(np.int16)

    return wrap16(sidx), wrap16(didx), gid, tiles, btot


def _build(tiles, btot, repeat=1, loop_repeat=1, no_compute=False, no_gather=False,
           bufs_g=8, nq=4, sq_dt=None, acc_dt=None, bufs_sq=3, lite=False,
           table_dt=None, scratch=16384, gprio=0):
    """Build the SPMD Bass program (one NEFF, all cores identical).

    repeat statically unrolls the tile pass; loop_repeat wraps it in an
    on-device For_i (used by test.py to measure steady-state HW time).
    no_compute / no_gather: diagnostic variants for isolating bottlenecks."""
    sq_dt = sq_dt or SQ_DT
    acc_dt = acc_dt or mybir.dt.float32
    table_dt = table_dt or mybir.dt.float32
    nc = bacc.Bacc("TRN2", target_bir_lowering=False, debug=False,
                   num_devices=NCORES, num_swdge_queues=nq,
                   dynamic_dma_scratch_size=scratch)
    h = nc.dram_tensor("h", [N, D], table_dt, kind="ExternalInput")
    sidx = nc.dram_tensor("sidx", [P, btot // 16], mybir.dt.int16, kind="ExternalInput")
    didx = nc.dram_tensor("didx", [P, btot // 16], mybir.dt.int16, kind="ExternalInput")
    out = nc.dram_tensor("out", [P, btot // P], mybir.dt.float32, kind="ExternalOutput")

    chunk_ap = [h[c * CH: min((c + 1) * CH, N), :] for c in range(NCHUNK)]

    with TileContext(nc) as tc:
        with (
            tc.tile_pool(name="idx", bufs=1) as idxp,
            tc.tile_pool(name="gat", bufs=bufs_g) as gp,
            tc.tile_pool(name="sq", bufs=bufs_sq) as sqp,
            tc.tile_pool(name="small", bufs=4) as smp,
            tc.tile_pool(name="cosb", bufs=1) as cosp,
        ):
            nc.gpsimd.load_library(library_config.mlp)
            si = idxp.tile([P, btot // 16], mybir.dt.int16)
            di = idxp.tile([P, btot // 16], mybir.dt.int16)
            nc.sync.dma_start(out=si[:], in_=sidx[:])
            nc.sync.dma_start(out=di[:], in_=didx[:])
            cosbuf = cosp.tile([P, btot // P], mybir.dt.float32)
            if no_compute:
                nc.gpsimd.memset(cosbuf[:], 0.0)
            else:
                dots = cosp.tile([P, btot // P], mybir.dt.float32)
                sss = cosp.tile([P, btot // P], acc_dt)
                sds = cosp.tile([P, btot // P], acc_dt)
                if lite:
                    nc.gpsimd.memset(dots[:], 1.0)
                    nc.gpsimd.memset(sss[:], 1.0)
                    nc.gpsimd.memset(sds[:], 1.0)
            if no_gather:
                gsP = idxp.tile([P, G // P, D], table_dt)
                gdP = idxp.tile([P, G // P, D], table_dt)
                nc.gpsimd.memset(gsP[:], 1.0)
                nc.gpsimd.memset(gdP[:], 1.0)

            from contextlib import nullcontext
            loop_ctx = (tc.For_i(0, loop_repeat, 1) if loop_repeat > 1
                        else nullcontext())
            with loop_ctx:
              for _ in range(repeat):
                for ti, (ca, cb, off, gsz) in enumerate(tiles):
                    m = gsz // P
                    io, c0 = off // 16, off // P
                    if no_gather:
                        gs, gd = gsP[:, :m], gdP[:, :m]
                    else:
                        gs = gp.tile([P, m, D], table_dt, tag="gs")
                        gd = gp.tile([P, m, D], table_dt, tag="gd")
                        from contextlib import nullcontext as _nc2
                        pctx = tc.high_priority(offset=gprio) if gprio else _nc2()
                        with pctx:
                            nc.gpsimd.dma_gather(gs[:], chunk_ap[ca],
                                                 si[:, io:io + gsz // 16],
                                                 gsz, gsz, D, single_packet=False,
                                                 queue_num=(2 * ti) % nq)
                            nc.gpsimd.dma_gather(gd[:], chunk_ap[cb],
                                                 di[:, io:io + gsz // 16],
                                                 gsz, gsz, D, single_packet=False,
                                                 queue_num=(2 * ti + 1) % nq)
                    if no_compute:
                        continue
                    prod = sqp.tile([P, m, D], mybir.dt.float32, tag="prod")
                    nc.vector._custom_dve(
                        MUL_SCAN,
                        out=prod[:].rearrange("p a b -> p (a b)"),
                        in0=gs[:].rearrange("p a b -> p (a b)"),
                        in1=gd[:].rearrange("p a b -> p (a b)"))
                    if lite == 2:      # scan only, no extraction
                        continue
                    if lite == 3:      # scan + single-col copy per tile
                        nc.vector.tensor_copy(out=dots[:, c0:c0 + 1],
                                              in_=prod[:, 0:1, D - 1])
                        continue
                    if lite == 4:      # scan + strided subtract per tile
                        if m > 1:
                            nc.vector.tensor_tensor(out=dots[:, c0 + 1:c0 + m],
                                                    in0=prod[:, 1:m, D - 1],
                                                    in1=prod[:, 0:m - 1, D - 1],
                                                    op=mybir.AluOpType.subtract)
                        continue
                    if lite == 5:      # scan + big reduce into persistent slice
                        nc.vector.tensor_reduce(sss[:, c0:c0 + m], gs[:],
                                                axis=mybir.AxisListType.X,
                                                op=mybir.AluOpType.add)
                        continue
                    if lite:
                        nc.vector.tensor_copy(out=dots[:, c0:c0 + 1],
                                              in_=prod[:, 0:1, D - 1])
                        if m > 1:
                            nc.vector.tensor_tensor(out=dots[:, c0 + 1:c0 + m],
                                                    in0=prod[:, 1:m, D - 1],
                                                    in1=prod[:, 0:m - 1, D - 1],
                                                    op=mybir.AluOpType.subtract)
                        nc.vector.tensor_copy(out=sss[:, c0:c0 + m], in_=dots[:, c0:c0 + m])
                        nc.vector.tensor_copy(out=sds[:, c0:c0 + m], in_=dots[:, c0:c0 + m])
                        continue
                    # squares in place (gs/gd no longer needed raw)
                    s2, d2 = gs, gd
                    nc.scalar.square(s2[:].rearrange("p a b -> p (a b)"),
                                     gs[:].rearrange("p a b -> p (a b)"))
                    nc.scalar.square(d2[:].rearrange("p a b -> p (a b)"),
                                     gd[:].rearrange("p a b -> p (a b)"))
                    nc.vector.tensor_copy(out=dots[:, c0:c0 + 1],
                                          in_=prod[:, 0:1, D - 1])
                    if m > 1:
                        nc.vector.tensor_tensor(out=dots[:, c0 + 1:c0 + m],
                                                in0=prod[:, 1:m, D - 1],
                                                in1=prod[:, 0:m - 1, D - 1],
                                                op=mybir.AluOpType.subtract)
                    from contextlib import nullcontext as _nullctx
                    lp = (nc.allow_low_precision(reason="norms in fp16 is accurate enough")
                          if acc_dt != mybir.dt.float32 else _nullctx())
                    with lp:
                        nc.vector.tensor_reduce(sss[:, c0:c0 + m], s2[:],
                                                axis=mybir.AxisListType.X,
                                                op=mybir.AluOpType.add)
                        nc.vector.tensor_reduce(sds[:, c0:c0 + m], d2[:],
                                                axis=mybir.AxisListType.X,
                                                op=mybir.AluOpType.add)
                if not no_compute:
                    # batched epilogue: cos = dot * rsqrt(ss*sd)
                    epi1 = cosp.tile([P, btot // P], mybir.dt.float32, tag="epi1")
                    epi2 = cosp.tile([P, btot // P], mybir.dt.float32, tag="epi2")
                    nc.vector.tensor_tensor(out=epi1[:], in0=sss[:], in1=sds[:],
                                            op=mybir.AluOpType.mult)
                    nc.scalar.sqrt(epi2[:], epi1[:])
                    nc.vector.reciprocal(epi1[:], epi2[:])
                    nc.vector.tensor_tensor(out=cosbuf[:], in0=dots[:], in1=epi1[:],
                                            op=mybir.AluOpType.mult)
            nc.sync.dma_start(out=out[:], in_=cosbuf[:])
    nc.compile()
    return nc


def kernel(h, src, dst):
    h = np.ascontiguousarray(np.asarray(h), dtype=np.float32)
    sidx_w, didx_w, gid, tiles, btot = _plan(src, dst)
    nc = _build(tiles, btot)
    in_maps = [
        {"h": h, "sidx": np.ascontiguousarray(sidx_w[c]),
         "didx": np.ascontiguousarray(didx_w[c])}
        for c in range(NCORES)
    ]
    res = bass_utils.run_bass_kernel_spmd(nc, in_maps, core_ids=list(range(NCORES)))
    full = np.zeros(E, np.float32)
    for c in range(NCORES):
        padded = res.results[c]["out"].T.ravel()     # padded-position order
        g = gid[c]
        valid = g >= 0
        full[g[valid]] = padded[valid]
    return full.reshape(E, 1)

